# revision 42
# baseline (speedup 1.0000x reference)
"""APLinear (4-bit bit-plane LUT-quantized GEMV) on 8 TRN2 NeuronCores.

out[o] = sum_i lut[o, idx[o,i]] * x[i], where idx is assembled from 4 bit-planes
packed MSB-first into int32 words (32 inputs per word).

Strategy (tensor-parallel over out_features, 1376 channels per core, padded to
1408 = 11 tiles of 128 partitions):
- Packed-domain two-level shift/mask combine of the 4 planes into 4
  nibble-interleaved K arrays (idx nibbles in packed int32 form), done in two
  chunks so tile 0 starts early.
- Extraction per tile: GPSIMD replicates K halfwords 4x into i-order (step-1
  int16 layout), DVE does an int16 LSR (2x mode) + AND 15 (4x mode), GPSIMD
  casts to fp16 idx [128, 4096].
- Histogram via single-source reduce-accumulation passes. On HW,
  tensor_scalar with accum_out computes accum = reduce_op1(in0 op0 s1)
  (op1 is the reduction operator, s2 seeds it), so per accumulator we get
  one functional sum_i f(elem). Families used (all linear in the 16 per-bin
  sums S_k = sum_{idx=k} x~ and counts N_k):
    R[5h+v] = sum_i max(z_h, v), v=0..4, z_h = idx + x~ - 4h fp16 (DVE 4x;
      z built on GPSIMD; R[19] is constant and folded on the host)
    A_k = sum_i relu(idx - k)  (ACT, k<N_ACT) / sum_i max(idx,k) (DVE, rest)
  Host folds the bin-unmixing and the lut into c2[o, 36]:
    out[o] = sum_j c2[o,j]*R[o,j],  R = [R_0..19, A_0..14, 4096].
- Host concatenates the 8 per-core output slices (column-parallel split).
"""

import sys

sys.path.insert(0, "/opt/trn_rl_repo")

import numpy as np
import concourse.bacc as bacc
import concourse.tile as tile
from concourse import mybir
from concourse.alu_op_type import AluOpType
from concourse.bass_utils import run_bass_kernel_spmd

IN = 4096
OUT = 11008
BW = 4
NCORES = 8
O_SHARD = OUT // NCORES  # 1376 (not a multiple of 128)
O_PAD = 1408  # padded to 11 tiles of 128
W = IN // 32  # 128 words per plane row
NT = O_PAD // 128  # 11 tiles
NTW = NT * W
N_ACT = 8  # count passes (A_k) on the ACT engine
N_POOL = 0  # GPSIMD tensor_reduce is partition-axis only; no Pool count passes

AND = AluOpType.bitwise_and
OR = AluOpType.bitwise_or
LSL = AluOpType.logical_shift_left
LSR = AluOpType.logical_shift_right

_cache = {}


def _s32(v):
    return int(np.uint32(v).view(np.int32)) if v > 0x7FFFFFFF else int(v)


def _build():
    nc = bacc.Bacc(None, target_bir_lowering=False, debug=False)

    qw_ext = nc.dram_tensor(
        "qweight", [BW, O_PAD, W], mybir.dt.int32, kind="ExternalInput"
    )
    lut_ext = nc.dram_tensor("lut", [O_PAD, 36], mybir.dt.float32, kind="ExternalInput")
    x_ext = nc.dram_tensor("x", [IN], mybir.dt.float32, kind="ExternalInput")
    out_ext = nc.dram_tensor("out", [O_PAD], mybir.dt.float32, kind="ExternalOutput")

    l_idx = np.repeat(np.arange(8), 4)
    shift16_np = np.tile((12 - 4 * (l_idx % 4)).astype(np.int16), W)
    shift16_dram = nc.inline_tensor(shift16_np, name="shift16_const")
    mu_dram = nc.inline_tensor(_mu_consts(), name="mu_const")

    with tile.TileContext(nc) as tc:
        with (
            tc.tile_pool(name="singles", bufs=1) as singles,
            tc.tile_pool(name="comb", bufs=1) as comb,
            tc.tile_pool(name="work", bufs=2) as work,
            tc.tile_pool(name="scratch", bufs=1) as scratch,
        ):
            planes_sb = comb.tile([128, BW, NT, W], mybir.dt.int32)
            lut_sb = singles.tile([128, NT, 36], mybir.dt.float32)
            xf32_sb = comb.tile([128, IN], mybir.dt.float32)
            xth_sb = singles.tile([128, 4, IN], mybir.dt.float16)
            shift16_sb = singles.tile([128, IN], mybir.dt.int16)
            Kbuf_sb = singles.tile([128, NTW, 4], mybir.dt.int32)
            D_sb = comb.tile([128, 4, NTW], mybir.dt.int32)
            A_sb = comb.tile([128, NTW], mybir.dt.int32)
            B_sb = comb.tile([128, NTW], mybir.dt.int32)
            ocol_sb = singles.tile([128, NT], mybir.dt.float32)

            nc.sync.dma_start(
                out=planes_sb[:],
                in_=qw_ext[:].rearrange("j (t p) w -> p j t w", p=128),
            )
            nc.sync.dma_start(
                out=lut_sb[:], in_=lut_ext[:].rearrange("(t p) v -> p t v", p=128)
            )
            nc.sync.dma_start(
                out=xf32_sb[:], in_=x_ext[:].unsqueeze(0).broadcast_to([128, IN])
            )
            nc.sync.dma_start(
                out=shift16_sb[:],
                in_=shift16_dram[:].unsqueeze(0).broadcast_to([128, IN]),
            )
            mu_sb = singles.tile([128, 36], mybir.dt.float32)
            nc.sync.dma_start(
                out=mu_sb[:],
                in_=mu_dram[:].unsqueeze(0).broadcast_to([128, 36]),
            )
            # x~ - 4h window arrays (fp16), h = 0..3
            nc.vector.tensor_copy(xth_sb[:, 0], xf32_sb[:])
            for h in range(1, 4):
                nc.vector.tensor_scalar(
                    xth_sb[:, h], xth_sb[:, 0], float(-4 * h), None, AluOpType.add
                )
            biasA_sb = singles.tile([128, N_ACT], mybir.dt.float32)
            for j in range(N_ACT):
                nc.vector.memset(biasA_sb[:, j : j + 1], float(-j))
            if N_POOL:
                ck_sb = singles.tile([128, N_POOL], mybir.dt.float16)
                for j in range(N_POOL):
                    nc.vector.memset(ck_sb[:, j : j + 1], float(15 - N_POOL + j))


            def ts(out, in0, s1, s2, op0, op1):
                nc.vector.tensor_scalar(out, in0, _s32(s1), s2, op0, op1)

            def combine(lo, hi):
                s = slice(lo * W, hi * W)
                pw = [
                    planes_sb[:, j, lo:hi, :].rearrange("p t w -> p (t w)")
                    for j in range(BW)
                ]
                A_t, B_t = A_sb[:, s], B_sb[:, s]
                D_t = [D_sb[:, j, s] for j in range(4)]
                kb = Kbuf_sb[:, s, :]
                # level 1: dibit combines (shift-then-mask on right shifts)
                ts(A_t, pw[0], 0x55555555, 1, AND, LSL)
                ts(B_t, pw[1], 0x55555555, 0, AND, LSL)
                nc.vector.tensor_tensor(D_t[0], A_t, B_t, OR)  # D_hi_e
                ts(A_t, pw[0], 0xAAAAAAAA, 0, AND, LSL)
                ts(B_t, pw[1], 1, 0x55555555, LSR, AND)
                nc.vector.tensor_tensor(D_t[1], A_t, B_t, OR)  # D_hi_o
                ts(A_t, pw[2], 0x55555555, 1, AND, LSL)
                ts(B_t, pw[3], 0x55555555, 0, AND, LSL)
                nc.vector.tensor_tensor(D_t[2], A_t, B_t, OR)  # D_lo_e
                ts(A_t, pw[2], 0xAAAAAAAA, 0, AND, LSL)
                ts(B_t, pw[3], 1, 0x55555555, LSR, AND)
                nc.vector.tensor_tensor(D_t[3], A_t, B_t, OR)  # D_lo_o
                # level 2: nibble combines -> Kbuf[:, :, q], sigma=[3,1,2,0]
                ts(A_t, D_t[0], 0x33333333, 2, AND, LSL)
                ts(B_t, D_t[2], 0x33333333, 0, AND, LSL)
                nc.vector.tensor_tensor(kb[:, :, 3], A_t, B_t, OR)  # K0
                ts(A_t, D_t[0], 0xCCCCCCCC, 0, AND, LSL)
                ts(B_t, D_t[2], 2, 0x33333333, LSR, AND)
                nc.vector.tensor_tensor(kb[:, :, 1], A_t, B_t, OR)  # K1
                ts(A_t, D_t[1], 0x33333333, 2, AND, LSL)
                ts(B_t, D_t[3], 0x33333333, 0, AND, LSL)
                nc.vector.tensor_tensor(kb[:, :, 2], A_t, B_t, OR)  # K2
                ts(A_t, D_t[1], 0xCCCCCCCC, 0, AND, LSL)
                ts(B_t, D_t[3], 2, 0x33333333, LSR, AND)
                nc.vector.tensor_tensor(kb[:, :, 0], A_t, B_t, OR)  # K3

            combine(0, 1)
            combine(1, 2)
            combine(2, NT)

            for t in range(NT):
                Srep_sb = scratch.tile([128, IN], mybir.dt.int16, tag="Srep")
                S16_sb = scratch.tile([128, IN], mybir.dt.int16, tag="S16")
                idx_sb = work.tile([128, IN], mybir.dt.float16, tag="idx")
                scr_sb = scratch.tile([128, IN], mybir.dt.float16, tag="scr")
                R_sb = work.tile([128, 36], mybir.dt.float32, tag="R")
                GL_sb = work.tile([128, 36], mybir.dt.float32, tag="GL")

                # Pool: replicate K halfwords 4x into i-order (hi then lo half)
                k16 = Kbuf_sb[:, t * W : (t + 1) * W, :].bitcast(mybir.dt.int16)
                # k16: [128, W, 8] (q-major pairs: [q0lo q0hi q1lo q1hi ...])
                rep_view = Srep_sb[:].rearrange(
                    "p (w l q) -> p w l q", w=W, l=8, q=4
                )
                hi_src = (
                    k16.rearrange("p w (q two) -> p w two q", q=4, two=2)[:, :, 1]
                    .unsqueeze(2)
                    .broadcast_to([128, W, 4, 4])
                )
                lo_src = (
                    k16.rearrange("p w (q two) -> p w two q", q=4, two=2)[:, :, 0]
                    .unsqueeze(2)
                    .broadcast_to([128, W, 4, 4])
                )
                nc.gpsimd.tensor_copy(rep_view[:, :, 0:4], hi_src)
                nc.gpsimd.tensor_copy(rep_view[:, :, 4:8], lo_src)
                # DVE: int16 LSR (2x) + AND (4x)
                nc.vector.tensor_tensor(
                    S16_sb[:], Srep_sb[:], shift16_sb[:], LSR
                )
                nc.vector.tensor_scalar(
                    S16_sb[:], S16_sb[:], 15, 0, AND, LSL
                )
                nc.gpsimd.tensor_copy(idx_sb[:], S16_sb[:])

                nc.vector.memset(R_sb[:], 0.0)
                nc.vector.memset(R_sb[:, 35:36], float(IN))
                scrA_sb = scratch.tile([128, IN], mybir.dt.float16, tag="scrA")
                # A_k on ACT (relu accumulate), k = 0..N_ACT-1
                for j in range(N_ACT):
                    nc.scalar.activation(
                        scrA_sb[:], idx_sb[:], mybir.ActivationFunctionType.Relu,
                        bias=biasA_sb[:, j : j + 1], scale=1.0,
                        accum_out=R_sb[:, 20 + j : 21 + j],
                    )
                # R[5h+v] = sum max(z_h, v) on DVE (4x fp16)
                for h in range(4):
                    z_sb = scratch.tile([128, IN], mybir.dt.float16, tag="z%d" % (h % 2))
                    nc.gpsimd.tensor_tensor(
                        z_sb[:], idx_sb[:], xth_sb[:, h], AluOpType.add
                    )
                    for v in range(5):
                        k = 5 * h + v
                        if k == 19:
                            continue  # R[19] = 4*N exactly (z_3 < 4); folded on host
                        nc.vector.tensor_scalar(
                            scr_sb[:], z_sb[:], float(v), 0.0,
                            AluOpType.max, AluOpType.add,
                            accum_out=R_sb[:, k : k + 1],
                        )
                # remaining counts: sum max(idx, k) = A_k + k*N
                # (DVE tensor_scalar accum; the last N_POOL via Pool TT+reduce)
                for k in range(N_ACT, 15 - N_POOL):
                    nc.vector.tensor_scalar(
                        scr_sb[:], idx_sb[:], float(k), 0.0,
                        AluOpType.max, AluOpType.add,
                        accum_out=R_sb[:, 20 + k : 21 + k],
                    )
                for k in range(15 - N_POOL, 15):
                    scrP_sb = scratch.tile([128, IN], mybir.dt.float16, tag="scrP")
                    nc.gpsimd.tensor_tensor(
                        scrP_sb[:],
                        idx_sb[:],
                        ck_sb[:, k - (15 - N_POOL) : k - (15 - N_POOL) + 1]
                        .broadcast_to([128, IN]),
                        AluOpType.max,
                    )
                    nc.gpsimd.tensor_reduce(
                        R_sb[:, 20 + k : 21 + k], scrP_sb[:],
                        mybir.AxisListType.X, AluOpType.add,
                    )
                nc.vector.tensor_tensor(
                    R_sb[:], R_sb[:], mu_sb[:], AluOpType.subtract
                )
                nc.vector.tensor_tensor(
                    GL_sb[:], R_sb[:], lut_sb[:, t, :], AluOpType.mult
                )
                nc.vector.tensor_reduce(
                    ocol_sb[:, t : t + 1], GL_sb[:], mybir.AxisListType.X,
                    AluOpType.add,
                )
            for t in range(NT):
                nc.sync.dma_start(
                    out=out_ext[t * 128 : (t + 1) * 128].unsqueeze(1),
                    in_=ocol_sb[:, t : t + 1],
                )
    nc.compile()
    return nc


def _get_nc():
    if "nc" not in _cache:
        _cache["nc"] = _build()
    return _cache["nc"]


def _mu_consts():
    """Data-independent centering constants for the 36 accumulator columns
    (assumes idx ~ uniform; any fixed centering kills the fp32 cancellation)."""
    N = float(IN)
    mu = np.zeros(36, np.float32)
    d = np.arange(16, dtype=np.float64)
    for h in range(4):
        for v in range(5):
            mu[5 * h + v] = N * np.maximum(d - 4 * h + 0.5, v).mean()
    for k in range(15):
        muA = N * np.maximum(d - k, 0).mean()
        if k >= N_ACT:
            muA += k * N  # DVE columns hold A_k + k*N
        mu[20 + k] = muA
    mu[35] = 0.0  # const column not centered
    return mu


def _coeffs(lut_s, a, sum_xt):
    """Build c2[o, 36]: out = sum_j c2[o,j]*R[o,j], R = [R_0..19, A_0..14, N].

    R[5h+v] = sum max(z_h, v); device A-col k holds A_k (ACT, k<N_ACT) or
    A_k + k*N (DVE max-family, k>=N_ACT).
    """
    O = lut_s.shape[0]
    N = float(IN)
    bR = np.zeros((O, 20), np.float64)
    bA = np.zeros((O, 15), np.float64)
    b0 = np.zeros(O, np.float64)

    def addA(kk, cf):
        if kk == -1:
            bA[:, 0] += cf
            b0[:] += cf * N
        elif 0 <= kk <= 14:
            bA[:, kk] += cf
        # kk == 15: A_15 = 0

    for k in range(16):
        lk = lut_s[:, k].astype(np.float64) / a
        h, v = k >> 2, k & 3
        bR[:, 5 * h + v] += lk
        bR[:, 5 * h + v + 1] += -lk
        b0 += lk * N
        for kk, cf in [(k, -1.0), (k + 1, 1.0)]:
            addA(kk, cf * lk)
        for kk, cf in [(k - 1, -0.5), (k, 1.0), (k + 1, -0.5)]:
            addA(kk, cf * lk)
    # fold dropped pass: R[19] = 4*N exactly (z_3 = idx-12+x~ < 4 always)
    b0 += bR[:, 19] * 4.0 * N
    bR[:, 19] = 0.0
    # DVE count columns hold A_k + k*N: correct b0
    for k in range(N_ACT, 15):
        b0 -= bA[:, k] * k * N
    c2 = np.zeros((O, 36), np.float64)
    c2[:, :20] = bR
    c2[:, 20:35] = bA
    c2[:, 35] = b0 / N
    # device subtracts mu from columns before the dot; fold sum(c*mu) back in
    mu = _mu_consts().astype(np.float64)
    c2[:, 35] += (c2[:, :35] * mu[None, :35]).sum(1) / N
    return np.ascontiguousarray(c2.astype(np.float32))


def _in_maps(x, qweight, lut):
    x64 = x.reshape(IN).astype(np.float64)
    xmax = np.abs(x64).max() + 1e-12
    a = 0.4995 / xmax
    x_f = np.ascontiguousarray((a * x64 + 0.5).astype(np.float32))
    # device x~ path: f32 -> fp16 copy; reproduce exactly for the R[0] fold
    sum_xt = float(x_f.astype(np.float16).astype(np.float64).sum())
    pad = O_PAD - O_SHARD
    maps = []
    for c in range(NCORES):
        sl = slice(c * O_SHARD, (c + 1) * O_SHARD)
        qw_s = np.ascontiguousarray(
            np.pad(qweight[:, sl, :], ((0, 0), (0, pad), (0, 0)))
        )
        lut_s = np.pad(lut[sl].astype(np.float64), ((0, pad), (0, 0)))
        c2 = _coeffs(lut_s, a, sum_xt)
        maps.append({"qweight": qw_s, "lut": np.ascontiguousarray(c2), "x": x_f})
    return maps


def run(x, qweight, lut, trace=False):
    nc = _get_nc()
    maps = _in_maps(x, qweight, lut)
    res = run_bass_kernel_spmd(nc, maps, core_ids=list(range(NCORES)), trace=trace)
    out = np.concatenate(
        [np.asarray(res.results[c]["out"]).reshape(-1)[:O_SHARD] for c in range(NCORES)]
    )
    out = out.reshape(1, 1, OUT).astype(np.float16)
    return out, res


def kernel(x, qweight, lut):
    out, _ = run(
        np.asarray(x), np.asarray(qweight), np.asarray(lut), trace=False
    )
    return out
